# revision 5
# baseline (speedup 1.0000x reference)
"""Causal single-head attention (B=16, T=2048, C=1024, H=64) on 8 TRN2 NeuronCores.

Strategy:
- Data-parallel over batch: 2 batches per core, weights replicated.
- Host passes x pre-transposed per batch (xT: [C, T]) so projections can
  contract over C on the PE partition dim with full-rate fp32r matmuls.
- Projections: packed [Wq.T | Wk.T] stationary -> QKT [128, T] (rows 0:64 = Q^T,
  64:128 = K^T); Wv.T -> VT [64, T]; V^T transposed to V natural via PE transpose.
- Attention computed transposed: S^T[k,q] = KT_blk.T @ QT (N=512 full rate),
  P' = exp(0.125*S^T) on ACT (no max subtraction needed: scores are O(1)),
  causal mask via precomputed 0/1 mask multiply on diagonal chunks,
  O'^T[65,q] = [V|1].T @ P' accumulated over k-chunks; row 64 = softmax denom.
- Final PE transpose back to natural layout, reciprocal + scale, DMA out.
"""
import os
import sys

for _p in ("/opt/trn_rl_repo", "/root/.axon_site/_ro/trn_rl_repo"):
    if os.path.isdir(_p) and _p not in sys.path:
        sys.path.insert(0, _p)

import numpy as np
import concourse.bacc as bacc
import concourse.mybir as mybir
from concourse.tile import TileContext
from concourse import bass_utils

F32 = mybir.dt.float32
F32R = mybir.dt.float32r
EXP = mybir.ActivationFunctionType.Exp

B, T, C, H = 16, 2048, 1024, 64
NCORES = 8
BPC = B // NCORES          # batches per core
NTS = T // 512             # 4 t/q slices of 512
NCH = C // 128             # 8 contraction chunks
NKC = T // 128             # 16 k chunks

LAST_EXEC_TIME_NS = None
LAST_RESULTS = None


def build():
    nc = bacc.Bacc(trn_type="TRN2")
    xt = nc.dram_tensor("xt", [BPC, C, T], F32R, kind="ExternalInput")
    wqk = nc.dram_tensor("wqk", [C, 128], F32R, kind="ExternalInput")
    wv = nc.dram_tensor("wv", [C, H], F32R, kind="ExternalInput")
    mask = nc.dram_tensor("mask", [128, 896], F32R, kind="ExternalInput")
    ident = nc.dram_tensor("ident", [128, 128], F32R, kind="ExternalInput")
    ones = nc.dram_tensor("ones", [128, NKC * 32], F32R, kind="ExternalInput")
    y = nc.dram_tensor("y", [BPC, T, H], F32, kind="ExternalOutput")

    with TileContext(nc) as tc:
        with tc.tile_pool(name="const", bufs=1) as const, \
             tc.tile_pool(name="xpool", bufs=2) as xpool, \
             tc.tile_pool(name="qktp", bufs=2) as qktp, \
             tc.tile_pool(name="vtp", bufs=2) as vtp, \
             tc.tile_pool(name="ktp", bufs=2) as ktp, \
             tc.tile_pool(name="vbigp", bufs=2) as vbigp, \
             tc.tile_pool(name="ptp", bufs=4) as ptp, \
             tc.tile_pool(name="osbp", bufs=2) as osbp, \
             tc.tile_pool(name="yp", bufs=4) as yp, \
             tc.tile_pool(name="ps512", bufs=4, space="PSUM") as ps512, \
             tc.tile_pool(name="pssm", bufs=4, space="PSUM") as pssm:

            wqk_sb = []
            wv_sb = []
            for c in range(NCH):
                wq_t = const.tile([128, 128], F32R, name=f"wqk{c}")
                nc.sync.dma_start(wq_t[:], wqk[128 * c:128 * (c + 1), :])
                wqk_sb.append(wq_t)
                wv_t = const.tile([128, H], F32R, name=f"wv{c}")
                nc.sync.dma_start(wv_t[:], wv[128 * c:128 * (c + 1), :])
                wv_sb.append(wv_t)
            mask_sb = const.tile([128, 896], F32R, name="mask_sb")
            nc.sync.dma_start(mask_sb[:], mask[:])
            id_sb = const.tile([128, 128], F32R, name="id_sb")
            nc.sync.dma_start(id_sb[:], ident[:])

            for b in range(BPC):
                qkt = qktp.tile([128, T], F32R, name="qkt", tag="qkt")
                vt = vtp.tile([64, T], F32R, name="vt", tag="vt")
                kt = ktp.tile([64, T], F32R, name="kt", tag="kt")
                vbig = vbigp.tile([128, NKC * 96], F32R, name="vbig", tag="vbig")
                vcols = vbig[:].rearrange("p (i c) -> p i c", c=96)[:, :, H:96]
                nc.sync.dma_start(vcols, ones[:].rearrange("p (i c) -> p i c", c=32))

                # ---- projections (contraction over C, streamed per t-slice) ----
                for ts in range(NTS):
                    xts = []
                    for c in range(NCH):
                        x_t = xpool.tile([128, 512], F32R, name=f"x{c}", tag=f"x{c}")
                        nc.sync.dma_start(
                            x_t[:], xt[b, 128 * c:128 * (c + 1), 512 * ts:512 * (ts + 1)])
                        xts.append(x_t)
                    qk_ps = ps512.tile([128, 512], F32, name="qk_ps", tag="ps512")
                    for c in range(NCH):
                        nc.tensor.matmul(qk_ps[:], wqk_sb[c][:], xts[c][:],
                                         start=(c == 0), stop=(c == NCH - 1))
                    nc.vector.tensor_copy(qkt[:, 512 * ts:512 * (ts + 1)], qk_ps[:])
                    nc.sync.dma_start(kt[:, 512 * ts:512 * (ts + 1)],
                                      qkt[64:128, 512 * ts:512 * (ts + 1)])
                    v_ps = pssm.tile([64, 512], F32, name="v_ps", tag="pssm")
                    for c in range(NCH):
                        nc.tensor.matmul(v_ps[:], wv_sb[c][:], xts[c][:],
                                         start=(c == 0), stop=(c == NCH - 1))
                    nc.vector.tensor_copy(vt[:, 512 * ts:512 * (ts + 1)], v_ps[:])

                # ---- V^T -> V natural (PE transpose per 128-token block) ----
                for i in range(NKC):
                    vtr_ps = pssm.tile([128, H], F32R, name="vtr_ps", tag="pssm")
                    nc.tensor.transpose(vtr_ps[:], vt[:, 128 * i:128 * (i + 1)],
                                        id_sb[0:64, 0:64])
                    nc.vector.tensor_copy(vbig[:, 96 * i:96 * i + H], vtr_ps[:])

                # ---- attention (transposed flash-style, causal) ----
                for j in range(NTS):
                    nck = 4 * j + 4
                    o_ps = pssm.tile([96, 512], F32, name="o_ps", tag="pssm")
                    for i in range(nck):
                        s_ps = ps512.tile([128, 512], F32, name="s_ps", tag="ps512")
                        nc.tensor.matmul(s_ps[:], kt[:, 128 * i:128 * (i + 1)],
                                         qkt[0:64, 512 * j:512 * (j + 1)],
                                         start=True, stop=True)
                        p_t = ptp.tile([128, 512], F32R, name="p_t", tag="pt")
                        nc.scalar.activation(p_t[:], s_ps[:], EXP, scale=0.125)
                        d = i - 4 * j
                        if d >= 0:
                            nc.vector.tensor_mul(
                                p_t[:], p_t[:],
                                mask_sb[:, 384 - 128 * d:896 - 128 * d])
                        nc.tensor.matmul(o_ps[:], vbig[:, 96 * i:96 * (i + 1)],
                                         p_t[:], start=(i == 0), stop=(i == nck - 1))
                    o_sb = osbp.tile([96, 512], F32R, name="o_sb", tag="osb")
                    nc.vector.tensor_copy(o_sb[:], o_ps[:])
                    for s in range(4):
                        f_ps = pssm.tile([128, 96], F32R, name="f_ps", tag="pssm")
                        nc.tensor.transpose(f_ps[:], o_sb[:, 128 * s:128 * (s + 1)],
                                            id_sb[0:96, 0:96])
                        rec = yp.tile([128, 1], F32, name="rec", tag="rec")
                        nc.vector.reciprocal(rec[:], f_ps[:, H:H + 1])
                        y_t = yp.tile([128, H], F32, name="y_t", tag="yt")
                        nc.vector.tensor_scalar_mul(y_t[:], f_ps[:, 0:H], rec[:])
                        q0 = 512 * j + 128 * s
                        nc.sync.dma_start(y[b, q0:q0 + 128, :], y_t[:])

    nc.finalize()
    return nc


_NC_CACHE = None


def _get_nc():
    global _NC_CACHE
    if _NC_CACHE is None:
        _NC_CACHE = build()
    return _NC_CACHE


def _make_mask():
    # mask[p, m] = 1.0 iff (m - 384) >= p ; diagonal chunk d uses cols
    # [384-128d : 896-128d) so mask[p, f] = (f - 128d >= p)
    p = np.arange(128)[:, None]
    m = np.arange(896)[None, :]
    return ((m - 384) >= p).astype(np.float32)


def kernel(x, Wk, Wq, Wv, _trace=False, _trace_kwargs=None):
    global LAST_EXEC_TIME_NS, LAST_RESULTS
    x = np.ascontiguousarray(np.asarray(x, dtype=np.float32))
    Wk = np.asarray(Wk, dtype=np.float32)
    Wq = np.asarray(Wq, dtype=np.float32)
    Wv = np.asarray(Wv, dtype=np.float32)

    wqk = np.ascontiguousarray(np.concatenate([Wq.T, Wk.T], axis=1))  # [C, 128]
    wv = np.ascontiguousarray(Wv.T)                                   # [C, H]
    mask = _make_mask()
    ident = np.eye(128, dtype=np.float32)
    ones_arr = np.zeros((128, NKC * 32), dtype=np.float32)
    ones_arr[:, 0::32] = 1.0

    in_maps = []
    for core in range(NCORES):
        xb = x[BPC * core:BPC * (core + 1)]                 # [2, T, C]
        xtb = np.ascontiguousarray(xb.transpose(0, 2, 1))   # [2, C, T]
        in_maps.append({"xt": xtb, "wqk": wqk, "wv": wv, "mask": mask,
                        "ident": ident, "ones": ones_arr})

    nc = _get_nc()
    kwargs = {}
    if _trace:
        kwargs["trace"] = True
        if _trace_kwargs:
            kwargs.update(_trace_kwargs)
    res = bass_utils.run_bass_kernel_spmd(nc, in_maps, core_ids=list(range(NCORES)),
                                          **kwargs)
    LAST_EXEC_TIME_NS = res.exec_time_ns
    LAST_RESULTS = res

    out = np.empty((B, T, H), dtype=np.float32)
    for core in range(NCORES):
        out[BPC * core:BPC * (core + 1)] = res.results[core]["y"]
    return out


# revision 6
# speedup vs baseline: 1.2545x; 1.2545x over previous
"""Causal single-head attention (B=16, T=2048, C=1024, H=64) on 8 TRN2 NeuronCores.

Strategy:
- Data-parallel over batch: 2 batches per core, weights replicated.
- Host passes x pre-transposed per batch (xT: [C, T]) so projections can
  contract over C on the PE partition dim with full-rate fp32r matmuls.
- Projections: packed [Wq.T | Wk.T] stationary -> QKT [128, T] (rows 0:64 = Q^T,
  64:128 = K^T); Wv.T -> VT [64, T]; V^T transposed to V natural via PE transpose.
- Attention computed transposed: S^T[k,q] = KT_blk.T @ QT (N=512 full rate),
  P' = exp(0.125*S^T) on ACT (no max subtraction needed: scores are O(1)),
  causal mask via precomputed 0/1 mask multiply on diagonal chunks,
  O'^T[65,q] = [V|1].T @ P' accumulated over k-chunks; row 64 = softmax denom.
- Final PE transpose back to natural layout, reciprocal + scale, DMA out.
"""
import os
import sys

for _p in ("/opt/trn_rl_repo", "/root/.axon_site/_ro/trn_rl_repo"):
    if os.path.isdir(_p) and _p not in sys.path:
        sys.path.insert(0, _p)

import numpy as np
import ml_dtypes
import concourse.bacc as bacc
import concourse.mybir as mybir
from concourse.tile import TileContext
from concourse import bass_utils

F32 = mybir.dt.float32
F32R = mybir.dt.float32r
BF16 = mybir.dt.bfloat16
EXP = mybir.ActivationFunctionType.Exp

B, T, C, H = 16, 2048, 1024, 64
NCORES = 8
BPC = B // NCORES          # batches per core
NTS = T // 512             # 4 t/q slices of 512
NCH = C // 128             # 8 contraction chunks
NKC = T // 128             # 16 k chunks

LAST_EXEC_TIME_NS = None
LAST_RESULTS = None


def build():
    nc = bacc.Bacc(trn_type="TRN2")
    xt = nc.dram_tensor("xt", [BPC, C, T], BF16, kind="ExternalInput")
    wqk = nc.dram_tensor("wqk", [C, 128], BF16, kind="ExternalInput")
    wv = nc.dram_tensor("wv", [C, H], BF16, kind="ExternalInput")
    mask = nc.dram_tensor("mask", [128, 896], BF16, kind="ExternalInput")
    ident = nc.dram_tensor("ident", [128, 128], F32R, kind="ExternalInput")
    ones = nc.dram_tensor("ones", [128, NKC * 32], BF16, kind="ExternalInput")
    y = nc.dram_tensor("y", [BPC, T, H], F32, kind="ExternalOutput")

    with TileContext(nc) as tc:
        with tc.tile_pool(name="const", bufs=1) as const, \
             tc.tile_pool(name="xpool", bufs=2) as xpool, \
             tc.tile_pool(name="qktp", bufs=2) as qktp, \
             tc.tile_pool(name="vtp", bufs=2) as vtp, \
             tc.tile_pool(name="ktp", bufs=2) as ktp, \
             tc.tile_pool(name="vbigp", bufs=2) as vbigp, \
             tc.tile_pool(name="ptp", bufs=4) as ptp, \
             tc.tile_pool(name="osbp", bufs=2) as osbp, \
             tc.tile_pool(name="yp", bufs=4) as yp, \
             tc.tile_pool(name="ps512", bufs=4, space="PSUM") as ps512, \
             tc.tile_pool(name="pssm", bufs=4, space="PSUM") as pssm:

            wqk_sb = []
            wv_sb = []
            for c in range(NCH):
                wq_t = const.tile([128, 128], BF16, name=f"wqk{c}")
                nc.sync.dma_start(wq_t[:], wqk[128 * c:128 * (c + 1), :])
                wqk_sb.append(wq_t)
                wv_t = const.tile([128, H], BF16, name=f"wv{c}")
                nc.sync.dma_start(wv_t[:], wv[128 * c:128 * (c + 1), :])
                wv_sb.append(wv_t)
            mask_sb = const.tile([128, 896], BF16, name="mask_sb")
            nc.sync.dma_start(mask_sb[:], mask[:])
            id_sb = const.tile([128, 128], F32R, name="id_sb")
            nc.sync.dma_start(id_sb[:], ident[:])

            for b in range(BPC):
                qkt = qktp.tile([128, T], BF16, name="qkt", tag="qkt")
                vt = vtp.tile([64, T], F32R, name="vt", tag="vt")
                kt = ktp.tile([64, T], BF16, name="kt", tag="kt")
                vbig = vbigp.tile([128, NKC * 96], BF16, name="vbig", tag="vbig")
                vcols = vbig[:].rearrange("p (i c) -> p i c", c=96)[:, :, H:96]
                nc.sync.dma_start(vcols, ones[:].rearrange("p (i c) -> p i c", c=32))

                # ---- projections (contraction over C, streamed per t-slice) ----
                for ts in range(NTS):
                    xts = []
                    for c in range(NCH):
                        x_t = xpool.tile([128, 512], BF16, name=f"x{c}", tag=f"x{c}")
                        nc.sync.dma_start(
                            x_t[:], xt[b, 128 * c:128 * (c + 1), 512 * ts:512 * (ts + 1)])
                        xts.append(x_t)
                    qk_ps = ps512.tile([128, 512], F32, name="qk_ps", tag="ps512")
                    for c in range(NCH):
                        nc.tensor.matmul(qk_ps[:], wqk_sb[c][:], xts[c][:],
                                         start=(c == 0), stop=(c == NCH - 1))
                    nc.vector.tensor_copy(qkt[:, 512 * ts:512 * (ts + 1)], qk_ps[:])
                    nc.sync.dma_start(kt[:, 512 * ts:512 * (ts + 1)],
                                      qkt[64:128, 512 * ts:512 * (ts + 1)])
                    v_ps = pssm.tile([64, 512], F32, name="v_ps", tag="pssm")
                    for c in range(NCH):
                        nc.tensor.matmul(v_ps[:], wv_sb[c][:], xts[c][:],
                                         start=(c == 0), stop=(c == NCH - 1))
                    nc.vector.tensor_copy(vt[:, 512 * ts:512 * (ts + 1)], v_ps[:])

                # ---- V^T -> V natural (PE transpose per 128-token block) ----
                for i in range(NKC):
                    vtr_ps = pssm.tile([128, H], F32R, name="vtr_ps", tag="pssm")
                    nc.tensor.transpose(vtr_ps[:], vt[:, 128 * i:128 * (i + 1)],
                                        id_sb[0:64, 0:64])
                    nc.vector.tensor_copy(vbig[:, 96 * i:96 * i + H], vtr_ps[:])

                # ---- attention (transposed flash-style, causal) ----
                for j in range(NTS):
                    nck = 4 * j + 4
                    o_ps = pssm.tile([96, 512], F32, name="o_ps", tag="pssm")
                    for i in range(nck):
                        s_ps = ps512.tile([128, 512], F32, name="s_ps", tag="ps512")
                        nc.tensor.matmul(s_ps[:], kt[:, 128 * i:128 * (i + 1)],
                                         qkt[0:64, 512 * j:512 * (j + 1)],
                                         start=True, stop=True)
                        p_t = ptp.tile([128, 512], BF16, name="p_t", tag="pt")
                        nc.scalar.activation(p_t[:], s_ps[:], EXP, scale=0.125)
                        d = i - 4 * j
                        if d >= 0:
                            nc.vector.tensor_mul(
                                p_t[:], p_t[:],
                                mask_sb[:, 384 - 128 * d:896 - 128 * d])
                        nc.tensor.matmul(o_ps[:], vbig[:, 96 * i:96 * (i + 1)],
                                         p_t[:], start=(i == 0), stop=(i == nck - 1))
                    o_sb = osbp.tile([96, 512], F32R, name="o_sb", tag="osb")
                    nc.vector.tensor_copy(o_sb[:], o_ps[:])
                    for s in range(4):
                        f_ps = pssm.tile([128, 96], F32R, name="f_ps", tag="pssm")
                        nc.tensor.transpose(f_ps[:], o_sb[:, 128 * s:128 * (s + 1)],
                                            id_sb[0:96, 0:96])
                        rec = yp.tile([128, 1], F32, name="rec", tag="rec")
                        nc.vector.reciprocal(rec[:], f_ps[:, H:H + 1])
                        y_t = yp.tile([128, H], F32, name="y_t", tag="yt")
                        nc.vector.tensor_scalar_mul(y_t[:], f_ps[:, 0:H], rec[:])
                        q0 = 512 * j + 128 * s
                        nc.sync.dma_start(y[b, q0:q0 + 128, :], y_t[:])

    nc.finalize()
    return nc


_NC_CACHE = None


def _get_nc():
    global _NC_CACHE
    if _NC_CACHE is None:
        _NC_CACHE = build()
    return _NC_CACHE


def _make_mask():
    # mask[p, m] = 1.0 iff (m - 384) >= p ; diagonal chunk d uses cols
    # [384-128d : 896-128d) so mask[p, f] = (f - 128d >= p)
    p = np.arange(128)[:, None]
    m = np.arange(896)[None, :]
    return ((m - 384) >= p).astype(np.float32)


def kernel(x, Wk, Wq, Wv, _trace=False, _trace_kwargs=None):
    global LAST_EXEC_TIME_NS, LAST_RESULTS
    x = np.ascontiguousarray(np.asarray(x, dtype=np.float32))
    Wk = np.asarray(Wk, dtype=np.float32)
    Wq = np.asarray(Wq, dtype=np.float32)
    Wv = np.asarray(Wv, dtype=np.float32)

    wqk = np.ascontiguousarray(
        np.concatenate([Wq.T, Wk.T], axis=1)).astype(ml_dtypes.bfloat16)  # [C, 128]
    wv = np.ascontiguousarray(Wv.T).astype(ml_dtypes.bfloat16)            # [C, H]
    mask = _make_mask().astype(ml_dtypes.bfloat16)
    ident = np.eye(128, dtype=np.float32)
    ones_arr = np.zeros((128, NKC * 32), dtype=ml_dtypes.bfloat16)
    ones_arr[:, 0::32] = 1.0

    in_maps = []
    for core in range(NCORES):
        xb = x[BPC * core:BPC * (core + 1)]                 # [2, T, C]
        xtb = np.ascontiguousarray(xb.transpose(0, 2, 1)).astype(ml_dtypes.bfloat16)
        in_maps.append({"xt": xtb, "wqk": wqk, "wv": wv, "mask": mask,
                        "ident": ident, "ones": ones_arr})

    nc = _get_nc()
    kwargs = {}
    if _trace:
        kwargs["trace"] = True
        if _trace_kwargs:
            kwargs.update(_trace_kwargs)
    res = bass_utils.run_bass_kernel_spmd(nc, in_maps, core_ids=list(range(NCORES)),
                                          **kwargs)
    LAST_EXEC_TIME_NS = res.exec_time_ns
    LAST_RESULTS = res

    out = np.empty((B, T, H), dtype=np.float32)
    for core in range(NCORES):
        out[BPC * core:BPC * (core + 1)] = res.results[core]["y"]
    return out
